# revision 6
# baseline (speedup 1.0000x reference)
"""Distributed GQA attention (B=2,S=2048,H=2048,NH=16,NKV=4,HD=128) on 8 TRN2 cores.

Strategy: tensor-parallel over heads (2 Q heads + 1 KV head per core).
K/V projections are split across core pairs by position (each core computes
its kv head's K and V for one batch only), exchanged with a pairwise
AllGather, with V produced transposed (512-wide matmuls) and fixed up by a
DMA transpose. Causal flash attention uses scores-transposed layout with
kt-paired exp instructions and pair-summed softmax denominators. An
AllToAll per head-half switches to sequence-parallel o_proj; the first
o_proj half is interleaved into the second attention half.
"""

import contextlib
import math

import numpy as np
import ml_dtypes

import concourse.bass as bass
import concourse.mybir as mybir
import concourse.tile as tile
from concourse.tile import add_dep_helper
from concourse import bacc
from concourse.bass_utils import run_bass_kernel_spmd
from concourse.masks import make_identity

BF16 = mybir.dt.bfloat16
F32 = mybir.dt.float32

B, S, H = 2, 2048, 2048
NH, NKV, HD = 16, 4, 128
NCORES = 8
HPC = NH // NCORES          # q heads per core = 2
POS = B * S                 # 4096 flattened rows
RPC = POS // NCORES         # output rows per core = 512
KT = H // 128               # 16 contraction tiles for projections
PT_N = POS // 512           # 8 pos-tiles of 512
HPT = PT_N // 2             # pos-tiles in my kv half = 4
SCALE = 1.0 / math.sqrt(HD)

_CACHE = {}


def _build():
    nc = bacc.Bacc("TRN2", target_bir_lowering=False, debug=False,
                   num_devices=NCORES)

    xT = nc.declare_dram_parameter("xT", [PT_N, KT, 128, 512], BF16,
                                   isOutput=False)
    xkv = nc.declare_dram_parameter("xkv", [HPT, KT, 128, 512], BF16,
                                    isOutput=False)
    wq = nc.declare_dram_parameter("wq", [KT, 128, HPC * HD], BF16,
                                   isOutput=False)
    wk = nc.declare_dram_parameter("wk", [KT, 128, HD], BF16, isOutput=False)
    wv = nc.declare_dram_parameter("wv", [KT, 128, HD], BF16, isOutput=False)
    cosT = nc.declare_dram_parameter("cosT", [HD, S], BF16, isOutput=False)
    ssinT = nc.declare_dram_parameter("ssinT", [HD, S], BF16, isOutput=False)
    wo = nc.declare_dram_parameter("wo", [KT, 128, H], BF16, isOutput=False)
    out = nc.declare_dram_parameter("out", [RPC, H], F32, isOutput=True)

    xT_t = xT.ap().rearrange("t k p n -> t p k n")
    xkv_t = xkv.ap().rearrange("t k p n -> t p k n")
    wq_t = wq.ap().rearrange("k p m -> p k m")
    wk_t = wk.ap().rearrange("k p m -> p k m")
    wv_t = wv.ap().rearrange("k p m -> p k m")
    wo_t = wo.ap().rearrange("k p m -> p k m")

    last_pe = [None]

    def pe(mm):
        # total order over PE instructions = emission order
        if last_pe[0] is not None:
            add_dep_helper(mm.ins, last_pe[0].ins, False)
        last_pe[0] = mm
        return mm

    with tile.TileContext(nc) as tc:
        with (
            tc.tile_pool(name="const", bufs=1) as const,
            tc.tile_pool(name="wpool", bufs=1) as wpool,
            tc.tile_pool(name="qkv", bufs=1) as qkv,
            tc.tile_pool(name="dram", bufs=1, space="DRAM") as dram,
        ):
            # ---- constants / weights resident in SBUF ----
            ident = const.tile([128, 128], BF16)
            make_identity(nc, ident)
            # upper-triangular (incl diag) mask: valid where kpos <= q
            triT = const.tile([128, 128], BF16)
            nc.gpsimd.memset(triT, 1.0)
            nc.gpsimd.affine_select(
                out=triT, in_=triT, compare_op=mybir.AluOpType.is_ge,
                fill=0.0, base=0, pattern=[[1, 128]], channel_multiplier=-1,
            )  # keep 1.0 where (c - p) >= 0, i.e. kpos <= q
            ones_sb = const.tile([128, 128], BF16)
            nc.gpsimd.memset(ones_sb, 1.0)

            cos_sb = const.tile([128, S], BF16)
            sin_sb = const.tile([128, S], BF16)
            cs_dmas = [nc.scalar.dma_start(cos_sb[:], cosT.ap()),
                       nc.scalar.dma_start(sin_sb[:], ssinT.ap())]

            wq_sb = wpool.tile([128, KT, HPC * HD], BF16)
            wk_sb = wpool.tile([128, KT, HD], BF16)
            wv_sb = wpool.tile([128, KT, HD], BF16)
            nc.scalar.dma_start(wk_sb[:], wk_t)
            nc.scalar.dma_start(wv_sb[:], wv_t)
            nc.scalar.dma_start(wq_sb[:], wq_t)
            wo_sb = wpool.tile([128, KT, H], BF16)
            at1_sb = wpool.tile([128, NCORES, RPC], BF16)
            at2_sb = wpool.tile([128, NCORES, RPC], BF16)

            # persistent q/k/v for both batches (bf16)
            q_all = qkv.tile([128, HPC, POS], BF16)
            kT_all = qkv.tile([128, POS], BF16)
            v_all = qkv.tile([128, POS // 128, HD], BF16)

            exch_in = dram.tile([128, 4096], BF16)
            exch_out = dram.tile([2, 128, 4096], BF16)
            a2a_in1 = dram.tile([NCORES, HD, RPC], BF16)
            a2a_out1 = dram.tile([NCORES, HD, RPC], BF16)
            a2a_in2 = dram.tile([NCORES, HD, RPC], BF16)
            a2a_out2 = dram.tile([NCORES, HD, RPC], BF16)

            # ---- PE warmup: flip HAM to K=8/8 before real matmuls ----
            with tc.tile_pool(name="psw", bufs=1, space="PSUM") as psw:
                ps_w = psw.tile([128, 128], F32, name="ps_w")
                for _ in range(90):
                    pe(nc.tensor.matmul(ps_w[:], ident[:], ident[:],
                                        start=True, stop=True))

            def rope(dst, ps, c0, rope_pool):
                """dst[128,512] bf16 = ps*cos + swap_halves(ps)*ssin."""
                ra = rope_pool.tile([128, 512], BF16, name="ra", tag="ra",
                                    bufs=3)
                rb = rope_pool.tile([128, 512], BF16, name="rb", tag="rb",
                                    bufs=3)
                nc.vector.tensor_tensor(
                    ra[:], ps[:], cos_sb[:, c0:c0 + 512], mybir.AluOpType.mult)
                nc.vector.tensor_tensor(
                    rb[0:64, :], ps[64:128, :], sin_sb[0:64, c0:c0 + 512],
                    mybir.AluOpType.mult)
                nc.vector.tensor_tensor(
                    rb[64:128, :], ps[0:64, :], sin_sb[64:128, c0:c0 + 512],
                    mybir.AluOpType.mult)
                nc.vector.tensor_tensor(dst, ra[:], rb[:],
                                        mybir.AluOpType.add)

            # ====== Phase A: K/V projection on my half + pair exchange ====
            with (
                tc.tile_pool(name="kvtiles", bufs=1) as kvtiles,
                tc.tile_pool(name="kvout", bufs=1) as kvout,
                tc.tile_pool(name="ropeA", bufs=1) as ropeA,
                tc.tile_pool(name="psA", bufs=1, space="PSUM") as psA,
            ):
                kTh = kvout.tile([128, S], BF16)
                vTh = kvout.tile([128, S], BF16)
                vh = kvout.tile([128, S // 128, HD], BF16)
                for pt in range(HPT):
                    c0 = pt * 512
                    xk_t = kvtiles.tile([128, KT, 512], BF16, name="xk_t",
                                        tag="xk", bufs=2)
                    for k8 in range(8):
                        xd = nc.sync.dma_start(
                            xk_t[:, k8 * 2:(k8 + 1) * 2, :],
                            xkv_t[pt, :, k8 * 2:(k8 + 1) * 2, :])
                    if pt == 0:
                        for csd in cs_dmas:
                            add_dep_helper(csd.ins, xd.ins, False)
                    ps_k = psA.tile([128, 512], F32, name="ps_k", tag="psk",
                                    bufs=2)
                    for k in range(KT):
                        pe(nc.tensor.matmul(ps_k[:], wk_sb[:, k, :],
                                            xk_t[:, k, :], start=(k == 0),
                                            stop=(k == KT - 1)))
                    rope(kTh[:, pt * 512:(pt + 1) * 512], ps_k, c0, ropeA)
                    ps_v = psA.tile([128, 512], F32, name="ps_v", tag="psv",
                                    bufs=2)
                    for k in range(KT):
                        pe(nc.tensor.matmul(ps_v[:], wv_sb[:, k, :],
                                            xk_t[:, k, :], start=(k == 0),
                                            stop=(k == KT - 1)))
                    nc.scalar.copy(vTh[:, pt * 512:(pt + 1) * 512], ps_v[:])

                # v: [d, pos] -> [pos%128, pos//128, d] via DMA transpose
                nc.scalar.dma_start_transpose(vh[:], vTh[:])
                nc.sync.dma_start(exch_in[:, 0:2048], kTh[:])
                nc.scalar.dma_start(exch_in[:, 2048:4096], vh[:])
                nc.gpsimd.collective_compute(
                    "AllGather", mybir.AluOpType.bypass,
                    replica_groups=[[0, 1], [2, 3], [4, 5], [6, 7]],
                    ins=[exch_in.opt()], outs=[exch_out.opt()])
                nc.sync.dma_start(kT_all[:, 0:2048], exch_out[0, :, 0:2048])
                nc.sync.dma_start(kT_all[:, 2048:4096],
                                  exch_out[1, :, 0:2048])
                nc.sync.dma_start(v_all[:, 0:16, :], exch_out[0, :, 2048:4096])
                nc.sync.dma_start(v_all[:, 16:32, :],
                                  exch_out[1, :, 2048:4096])

            # ====== Phase B: Q projection (global order) =================
            with (
                tc.tile_pool(name="att", bufs=1) as att,
                tc.tile_pool(name="ps2", bufs=1, space="PSUM") as ps2,
            ):
                qstack = contextlib.ExitStack()
                xtiles = qstack.enter_context(
                    tc.tile_pool(name="xtiles", bufs=1))
                ropeB = qstack.enter_context(tc.tile_pool(name="ropeB", bufs=1))
                psB = qstack.enter_context(
                    tc.tile_pool(name="psB", bufs=1, space="PSUM"))
                pending = []
                first_att_mm = [None]

                def q_proj(pt):
                    c0 = (pt * 512) % S
                    x_t = xtiles.tile([128, KT, 512], BF16, name="x_t",
                                      tag="x", bufs=2)
                    for k4 in range(4):
                        nc.sync.dma_start(
                            x_t[:, k4 * 4:(k4 + 1) * 4, :],
                            xT_t[pt, :, k4 * 4:(k4 + 1) * 4, :])
                    for hh in range(HPC):
                        ps_q = psB.tile([128, 512], F32, name="ps_q",
                                        tag="psq", bufs=2)
                        for k in range(KT):
                            mm = pe(nc.tensor.matmul(
                                ps_q[:], wq_sb[:, k, hh * 128:(hh + 1) * 128],
                                x_t[:, k, :], start=(k == 0),
                                stop=(k == KT - 1)))
                            for p in pending:
                                add_dep_helper(mm.ins, p.ins, False)
                            pending.clear()
                        rope(q_all[:, hh, pt * 512:(pt + 1) * 512], ps_q,
                             c0, ropeB)

                # ====== attention unit (ST flash, kt-paired exp) =========
                def attn_unit(hh, b, qsb):
                    qT = q_all[:, hh, b * S:(b + 1) * S]
                    kTb = kT_all[:, b * S:(b + 1) * S]
                    voff = b * (S // 128)
                    qs = qsb * 512
                    a2a_in = a2a_in1 if hh == 0 else a2a_in2
                    o_ps = ps2.tile([128, 512], F32, name="o_ps", tag="ops",
                                    bufs=1)
                    sum_ps = ps2.tile([128, 512], F32, name="sum_ps",
                                      tag="sums", bufs=1)
                    nkt = 4 * qsb + 4
                    for pr in range(nkt // 2):
                        kt0, kt1 = 2 * pr, 2 * pr + 1
                        jj0, jj1 = kt0 - 4 * qsb, kt1 - 4 * qsb
                        c00 = 0 if jj0 < 0 else jj0 * 128
                        c01 = 0 if jj1 < 0 else jj1 * 128
                        st = ps2.tile([128, 2, 512], F32, name="st",
                                      tag="stp", bufs=2)
                        mm = pe(nc.tensor.matmul(
                            st[:, 0, c00:512],
                            kTb[:, kt0 * 128:(kt0 + 1) * 128],
                            qT[:, qs + c00:qs + 512], start=True, stop=True))
                        if first_att_mm[0] is None:
                            first_att_mm[0] = mm
                        for p in pending:
                            add_dep_helper(mm.ins, p.ins, False)
                        pending.clear()
                        pe(nc.tensor.matmul(
                            st[:, 1, c01:512],
                            kTb[:, kt1 * 128:(kt1 + 1) * 128],
                            qT[:, qs + c01:qs + 512], start=True, stop=True))
                        pt_sb = att.tile([128, 2, 512], BF16, name="pt_sb",
                                         tag="ptp", bufs=4)
                        nc.scalar.activation(
                            pt_sb[:, :, c00:512], st[:, :, c00:512],
                            mybir.ActivationFunctionType.Exp, scale=SCALE)
                        if jj0 >= 0:
                            nc.vector.tensor_tensor(
                                pt_sb[:, 0, jj0 * 128:(jj0 + 1) * 128],
                                pt_sb[:, 0, jj0 * 128:(jj0 + 1) * 128],
                                triT[:], mybir.AluOpType.mult)
                            nc.vector.tensor_tensor(
                                pt_sb[:, 1, jj1 * 128:(jj1 + 1) * 128],
                                pt_sb[:, 1, jj1 * 128:(jj1 + 1) * 128],
                                triT[:], mybir.AluOpType.mult)
                        # pair-sum for the softmax denominator
                        padd = att.tile([128, 512], BF16, name="padd",
                                        tag="padd", bufs=2)
                        if jj0 < 0:
                            nc.vector.tensor_tensor(
                                padd[:], pt_sb[:, 0, :], pt_sb[:, 1, :],
                                mybir.AluOpType.add)
                        else:
                            nc.vector.tensor_copy(
                                padd[:, c00:c01], pt_sb[:, 0, c00:c01])
                            nc.vector.tensor_tensor(
                                padd[:, c01:512], pt_sb[:, 0, c01:512],
                                pt_sb[:, 1, c01:512], mybir.AluOpType.add)
                        pe(nc.tensor.matmul(
                            sum_ps[:, c00:512], ones_sb[:], padd[:, c00:512],
                            start=(pr == 0), stop=(pr == nkt // 2 - 1)))
                        pe(nc.tensor.matmul(
                            o_ps[:, c00:512], v_all[:, voff + kt0, :],
                            pt_sb[:, 0, c00:512], start=(kt0 == 0),
                            stop=False))
                        pe(nc.tensor.matmul(
                            o_ps[:, c01:512], v_all[:, voff + kt1, :],
                            pt_sb[:, 1, c01:512], start=False,
                            stop=(kt1 == nkt - 1)))

                    recip = att.tile([128, 512], F32, name="recip",
                                     tag="recip", bufs=2)
                    nc.vector.reciprocal_approx_fast(recip[:], sum_ps[:])
                    oT_sb = att.tile([128, 512], BF16, name="oT_sb",
                                     tag="osb", bufs=2)
                    nc.vector.scalar_tensor_tensor(
                        oT_sb[:], o_ps[:], 1.0, recip[:],
                        mybir.AluOpType.mult, mybir.AluOpType.mult)
                    d = nc.sync.dma_start(a2a_in[b * 4 + qsb, :, :], oT_sb[:])
                    pending.append(d)

                # ---- emission schedule ----
                for pt in range(4):
                    q_proj(pt)
                units_h0 = [(0, 0, qsb) for qsb in range(4)]
                for i, u in enumerate(units_h0):
                    attn_unit(*u)
                    q_proj(4 + i)
                qstack.close()   # free x/rope/psq space before o_proj pools
                for qsb in range(4):
                    attn_unit(0, 1, qsb)
                nc.gpsimd.collective_compute(
                    "AllToAll", mybir.AluOpType.bypass,
                    replica_groups=[list(range(NCORES))],
                    ins=[a2a_in1.opt()], outs=[a2a_out1.opt()])

                # ====== Phase 3 part 1 (h0 contraction) interleaved ======
                with (
                    tc.tile_pool(name="proj", bufs=1) as proj,
                    tc.tile_pool(name="ps3", bufs=1, space="PSUM") as ps3,
                ):
                    for k4 in range(4):
                        wd = nc.scalar.dma_start(
                            wo_sb[:, k4 * 4:(k4 + 1) * 4, :],
                            wo_t[:, k4 * 4:(k4 + 1) * 4, :])
                        add_dep_helper(wd.ins, first_att_mm[0].ins, False)
                    for r in range(NCORES):
                        nc.sync.dma_start(at1_sb[:, r, :], a2a_out1[r, :, :])
                    s1_sb = proj.tile([128, 16, 512], F32)

                    def part1_unit(ti):
                        mp, nn = ti // 4, ti % 4
                        ps_a = ps3.tile([128, 512], F32, name="ps_a",
                                        tag="po", bufs=2)
                        for r in range(NCORES):
                            pe(nc.tensor.matmul(
                                ps_a[:],
                                at1_sb[:, r, mp * 128:(mp + 1) * 128],
                                wo_sb[:, 2 * r, nn * 512:(nn + 1) * 512],
                                start=(r == 0), stop=(r == NCORES - 1)))
                        nc.vector.tensor_copy(s1_sb[:, ti, :], ps_a[:])

                    ti = 0
                    for b in range(B):
                        for qsb in range(4):
                            attn_unit(1, b, qsb)
                            part1_unit(ti)
                            part1_unit(ti + 1)
                            ti += 2
                    nc.gpsimd.collective_compute(
                        "AllToAll", mybir.AluOpType.bypass,
                        replica_groups=[list(range(NCORES))],
                        ins=[a2a_in2.opt()], outs=[a2a_out2.opt()])

                    # ====== Phase 3 part 2 (h1 contraction) + output =====
                    for r in range(NCORES):
                        nc.sync.dma_start(at2_sb[:, r, :], a2a_out2[r, :, :])
                    for mp in range(RPC // 128):
                        for nn in range(H // 512):
                            ti = mp * 4 + nn
                            ps_b = ps3.tile([128, 512], F32, name="ps_b",
                                            tag="po", bufs=2)
                            for r in range(NCORES):
                                mm = pe(nc.tensor.matmul(
                                    ps_b[:],
                                    at2_sb[:, r, mp * 128:(mp + 1) * 128],
                                    wo_sb[:, 2 * r + 1,
                                          nn * 512:(nn + 1) * 512],
                                    start=(r == 0), stop=(r == NCORES - 1)))
                                for p in pending:
                                    add_dep_helper(mm.ins, p.ins, False)
                                pending.clear()
                            ev = proj.tile([128, 512], F32, name="ev",
                                           tag="ev", bufs=3)
                            nc.vector.scalar_tensor_tensor(
                                ev[:], ps_b[:], 1.0, s1_sb[:, ti, :],
                                mybir.AluOpType.mult, mybir.AluOpType.add)
                            nc.sync.dma_start(
                                out.ap()[mp * 128:(mp + 1) * 128,
                                         nn * 512:(nn + 1) * 512], ev[:])

    nc.compile()
    return nc


def _get_nc():
    if "nc" not in _CACHE:
        _CACHE["nc"] = _build()
    return _CACHE["nc"]


def _prep_inputs(x, cos, sin, wq, wk, wv, wo):
    bf = ml_dtypes.bfloat16
    xf = np.asarray(x, np.float32).reshape(POS, H)
    # [PT_N, KT, 128, 512]: xTt[pt,k,p,j] = x[pt*512+j, k*128+p]
    xT = np.ascontiguousarray(
        xf.reshape(PT_N, 512, KT, 128).transpose(0, 2, 3, 1)).astype(bf)
    cosT = np.ascontiguousarray(np.asarray(cos, np.float32).T).astype(bf)
    sinT = np.asarray(sin, np.float32).T.copy()
    sinT[0:64, :] = -sinT[0:64, :]
    sinT = np.ascontiguousarray(sinT).astype(bf)
    wo_b = np.ascontiguousarray(
        np.asarray(wo, np.float32).reshape(KT, 128, H)).astype(bf)
    wq = np.asarray(wq, np.float32)
    wk = np.asarray(wk, np.float32)
    wv = np.asarray(wv, np.float32)

    in_maps = []
    for i in range(NCORES):
        kv = i // 2
        half = i % 2
        in_maps.append({
            "xT": xT,
            "xkv": np.ascontiguousarray(xT[half * HPT:(half + 1) * HPT]),
            "wq": np.ascontiguousarray(
                wq[:, i * HPC * HD:(i + 1) * HPC * HD].reshape(
                    KT, 128, HPC * HD)).astype(bf),
            "wk": np.ascontiguousarray(
                wk[:, kv * HD:(kv + 1) * HD].reshape(KT, 128, HD)).astype(bf),
            "wv": np.ascontiguousarray(
                wv[:, kv * HD:(kv + 1) * HD].reshape(KT, 128, HD)).astype(bf),
            "cosT": cosT,
            "ssinT": sinT,
            "wo": wo_b,
        })
    return in_maps


def kernel(x, cos, sin, wq, wk, wv, wo, _trace=False):
    nc = _get_nc()
    in_maps = _prep_inputs(x, cos, sin, wq, wk, wv, wo)
    res = run_bass_kernel_spmd(nc, in_maps, core_ids=list(range(NCORES)),
                               trace=_trace)
    rows = np.concatenate([np.asarray(res.results[i]["out"])
                           for i in range(NCORES)], axis=0)
    out = rows.reshape(B, S, H).astype(np.float32)
    if _trace:
        _CACHE["last_exec_time_ns"] = res.exec_time_ns
        _CACHE["last_results"] = res
    return out


# revision 8
# speedup vs baseline: 1.5292x; 1.5292x over previous
"""Distributed GQA attention (B=2,S=2048,H=2048,NH=16,NKV=4,HD=128) on 8 TRN2 cores.

Strategy: tensor-parallel over heads (2 Q heads + 1 KV head per core).
K/V projections are split across core pairs by position (each core computes
its kv head's K and V for one batch only), exchanged with a pairwise
AllGather, with V produced transposed (512-wide matmuls) and fixed up by a
DMA transpose. Causal flash attention uses scores-transposed layout with
kt-paired exp instructions and pair-summed softmax denominators. An
AllToAll per head-half switches to sequence-parallel o_proj; the first
o_proj half is interleaved into the second attention half.
"""

import contextlib
import math

import numpy as np
import ml_dtypes

import concourse.bass as bass
import concourse.mybir as mybir
import concourse.tile as tile
from concourse.tile import add_dep_helper
from concourse import bacc
from concourse.bass_utils import run_bass_kernel_spmd
from concourse.masks import make_identity

BF16 = mybir.dt.bfloat16
F32 = mybir.dt.float32

B, S, H = 2, 2048, 2048
NH, NKV, HD = 16, 4, 128
NCORES = 8
HPC = NH // NCORES          # q heads per core = 2
POS = B * S                 # 4096 flattened rows
RPC = POS // NCORES         # output rows per core = 512
KT = H // 128               # 16 contraction tiles for projections
PT_N = POS // 512           # 8 pos-tiles of 512
HPT = PT_N // 2             # pos-tiles in my kv half = 4
SCALE = 1.0 / math.sqrt(HD)

_CACHE = {}


def _build():
    nc = bacc.Bacc("TRN2", target_bir_lowering=False, debug=False,
                   num_devices=NCORES)

    xT = nc.declare_dram_parameter("xT", [PT_N, KT, 128, 512], BF16,
                                   isOutput=False)
    xkv = nc.declare_dram_parameter("xkv", [HPT, KT, 128, 512], BF16,
                                    isOutput=False)
    wq = nc.declare_dram_parameter("wq", [KT, 128, HPC * HD], BF16,
                                   isOutput=False)
    wk = nc.declare_dram_parameter("wk", [KT, 128, HD], BF16, isOutput=False)
    wv = nc.declare_dram_parameter("wv", [KT, 128, HD], BF16, isOutput=False)
    cosT = nc.declare_dram_parameter("cosT", [HD, S], BF16, isOutput=False)
    ssinT = nc.declare_dram_parameter("ssinT", [HD, S], BF16, isOutput=False)
    wo = nc.declare_dram_parameter("wo", [KT, 128, H], BF16, isOutput=False)
    out = nc.declare_dram_parameter("out", [RPC, H], F32, isOutput=True)

    xT_t = xT.ap().rearrange("t k p n -> t p k n")
    xkv_t = xkv.ap().rearrange("t k p n -> t p k n")
    wq_t = wq.ap().rearrange("k p m -> p k m")
    wk_t = wk.ap().rearrange("k p m -> p k m")
    wv_t = wv.ap().rearrange("k p m -> p k m")
    wo_t = wo.ap().rearrange("k p m -> p k m")

    unit_last = [None]
    unit_first = [None]
    unit_latest = [None]

    def pe(mm):
        # chain PE work at unit granularity: the first matmul of each unit
        # depends on the last matmul of the previous unit; within a unit the
        # scheduler is free to pipeline.
        if unit_first[0] is None:
            unit_first[0] = mm
            if unit_last[0] is not None:
                add_dep_helper(mm.ins, unit_last[0].ins, False)
        unit_latest[0] = mm
        return mm

    def close_unit():
        unit_last[0] = unit_latest[0]
        unit_first[0] = None

    with tile.TileContext(nc) as tc:
        with (
            tc.tile_pool(name="const", bufs=1) as const,
            tc.tile_pool(name="wpool", bufs=1) as wpool,
            tc.tile_pool(name="qkv", bufs=1) as qkv,
            tc.tile_pool(name="dram", bufs=1, space="DRAM") as dram,
        ):
            # ---- constants / weights resident in SBUF ----
            ident = const.tile([128, 128], BF16)
            make_identity(nc, ident)
            # upper-triangular (incl diag) mask: valid where kpos <= q
            triT = const.tile([128, 128], BF16)
            nc.gpsimd.memset(triT, 1.0)
            nc.gpsimd.affine_select(
                out=triT, in_=triT, compare_op=mybir.AluOpType.is_ge,
                fill=0.0, base=0, pattern=[[1, 128]], channel_multiplier=-1,
            )  # keep 1.0 where (c - p) >= 0, i.e. kpos <= q
            ones_sb = const.tile([128, 128], BF16)
            nc.gpsimd.memset(ones_sb, 1.0)

            cos_sb = const.tile([128, S], BF16)
            sin_sb = const.tile([128, S], BF16)
            cs_dmas = [nc.scalar.dma_start(cos_sb[:], cosT.ap()),
                       nc.scalar.dma_start(sin_sb[:], ssinT.ap())]

            wq_sb = wpool.tile([128, KT, HPC * HD], BF16)
            wk_sb = wpool.tile([128, KT, HD], BF16)
            wv_sb = wpool.tile([128, KT, HD], BF16)
            nc.scalar.dma_start(wk_sb[:], wk_t)
            nc.scalar.dma_start(wv_sb[:], wv_t)
            nc.scalar.dma_start(wq_sb[:], wq_t)
            wo_sb = wpool.tile([128, KT, H], BF16)
            at1_sb = wpool.tile([128, NCORES, RPC], BF16)
            at2_sb = wpool.tile([128, NCORES, RPC], BF16)

            # persistent q/k/v for both batches (bf16)
            q_all = qkv.tile([128, HPC, POS], BF16)
            kT_all = qkv.tile([128, POS], BF16)
            v_all = qkv.tile([128, POS // 128, HD], BF16)

            exch_in = dram.tile([128, 4096], BF16)
            exch_out = dram.tile([2, 128, 4096], BF16)
            a2a_in1 = dram.tile([NCORES, HD, RPC], BF16)
            a2a_out1 = dram.tile([NCORES, HD, RPC], BF16)
            a2a_in2 = dram.tile([NCORES, HD, RPC], BF16)
            a2a_out2 = dram.tile([NCORES, HD, RPC], BF16)

            # ---- PE warmup: flip HAM to K=8/8 before real matmuls ----
            with tc.tile_pool(name="psw", bufs=1, space="PSUM") as psw:
                ps_w = psw.tile([128, 128], F32, name="ps_w")
                for _ in range(90):
                    pe(nc.tensor.matmul(ps_w[:], ident[:], ident[:],
                                        start=True, stop=True))
                close_unit()

            def rope(dst, ps, c0, rope_pool):
                """dst[128,512] bf16 = ps*cos + swap_halves(ps)*ssin."""
                ra = rope_pool.tile([128, 512], BF16, name="ra", tag="ra",
                                    bufs=3)
                rb = rope_pool.tile([128, 512], BF16, name="rb", tag="rb",
                                    bufs=3)
                nc.vector.tensor_tensor(
                    ra[:], ps[:], cos_sb[:, c0:c0 + 512], mybir.AluOpType.mult)
                nc.vector.tensor_tensor(
                    rb[0:64, :], ps[64:128, :], sin_sb[0:64, c0:c0 + 512],
                    mybir.AluOpType.mult)
                nc.vector.tensor_tensor(
                    rb[64:128, :], ps[0:64, :], sin_sb[64:128, c0:c0 + 512],
                    mybir.AluOpType.mult)
                nc.vector.tensor_tensor(dst, ra[:], rb[:],
                                        mybir.AluOpType.add)

            # ====== Phase A: K/V projection on my half + pair exchange ====
            with (
                tc.tile_pool(name="kvtiles", bufs=1) as kvtiles,
                tc.tile_pool(name="kvout", bufs=1) as kvout,
                tc.tile_pool(name="ropeA", bufs=1) as ropeA,
                tc.tile_pool(name="psA", bufs=1, space="PSUM") as psA,
            ):
                kTh = kvout.tile([128, S], BF16)
                vTh = kvout.tile([128, S], BF16)
                vh = kvout.tile([128, S // 128, HD], BF16)
                for pt in range(HPT):
                    c0 = pt * 512
                    xk_t = kvtiles.tile([128, KT, 512], BF16, name="xk_t",
                                        tag="xk", bufs=2)
                    for k8 in range(8):
                        xd = nc.sync.dma_start(
                            xk_t[:, k8 * 2:(k8 + 1) * 2, :],
                            xkv_t[pt, :, k8 * 2:(k8 + 1) * 2, :])
                    if pt == 0:
                        for csd in cs_dmas:
                            add_dep_helper(csd.ins, xd.ins, False)
                    ps_k = psA.tile([128, 512], F32, name="ps_k", tag="psk",
                                    bufs=2)
                    for k in range(KT):
                        pe(nc.tensor.matmul(ps_k[:], wk_sb[:, k, :],
                                            xk_t[:, k, :], start=(k == 0),
                                            stop=(k == KT - 1)))
                    close_unit()
                    rope(kTh[:, pt * 512:(pt + 1) * 512], ps_k, c0, ropeA)
                    ps_v = psA.tile([128, 512], F32, name="ps_v", tag="psv",
                                    bufs=2)
                    for k in range(KT):
                        pe(nc.tensor.matmul(ps_v[:], wv_sb[:, k, :],
                                            xk_t[:, k, :], start=(k == 0),
                                            stop=(k == KT - 1)))
                    close_unit()
                    nc.scalar.copy(vTh[:, pt * 512:(pt + 1) * 512], ps_v[:])

                # v: [d, pos] -> [pos%128, pos//128, d] via DMA transpose
                nc.scalar.dma_start_transpose(vh[:], vTh[:])
                nc.scalar.dma_start(exch_in[:, 0:2048], kTh[:])
                nc.scalar.dma_start(exch_in[:, 2048:4096], vh[:])
                nc.gpsimd.collective_compute(
                    "AllGather", mybir.AluOpType.bypass,
                    replica_groups=[[0, 1], [2, 3], [4, 5], [6, 7]],
                    ins=[exch_in.opt()], outs=[exch_out.opt()])
                nc.scalar.dma_start(kT_all[:, 0:2048], exch_out[0, :, 0:2048])
                nc.scalar.dma_start(kT_all[:, 2048:4096],
                                  exch_out[1, :, 0:2048])
                nc.scalar.dma_start(v_all[:, 0:16, :], exch_out[0, :, 2048:4096])
                nc.scalar.dma_start(v_all[:, 16:32, :],
                                  exch_out[1, :, 2048:4096])

            # ====== Phase B: Q projection (global order) =================
            with (
                tc.tile_pool(name="att", bufs=1) as att,
                tc.tile_pool(name="ps2", bufs=1, space="PSUM") as ps2,
            ):
                qstack = contextlib.ExitStack()
                xtiles = qstack.enter_context(
                    tc.tile_pool(name="xtiles", bufs=1))
                ropeB = qstack.enter_context(tc.tile_pool(name="ropeB", bufs=1))
                psB = qstack.enter_context(
                    tc.tile_pool(name="psB", bufs=1, space="PSUM"))
                pending = []
                first_att_mm = [None]

                def q_proj(pt):
                    c0 = (pt * 512) % S
                    x_t = xtiles.tile([128, KT, 512], BF16, name="x_t",
                                      tag="x", bufs=2)
                    for k4 in range(4):
                        nc.sync.dma_start(
                            x_t[:, k4 * 4:(k4 + 1) * 4, :],
                            xT_t[pt, :, k4 * 4:(k4 + 1) * 4, :])
                    for hh in range(HPC):
                        ps_q = psB.tile([128, 512], F32, name="ps_q",
                                        tag="psq", bufs=2)
                        for k in range(KT):
                            mm = pe(nc.tensor.matmul(
                                ps_q[:], wq_sb[:, k, hh * 128:(hh + 1) * 128],
                                x_t[:, k, :], start=(k == 0),
                                stop=(k == KT - 1)))
                            for p in pending:
                                add_dep_helper(mm.ins, p.ins, False)
                            pending.clear()
                        close_unit()
                        rope(q_all[:, hh, pt * 512:(pt + 1) * 512], ps_q,
                             c0, ropeB)

                # ====== attention unit (ST flash, kt-paired exp) =========
                def attn_unit(hh, b, qsb):
                    qT = q_all[:, hh, b * S:(b + 1) * S]
                    kTb = kT_all[:, b * S:(b + 1) * S]
                    voff = b * (S // 128)
                    qs = qsb * 512
                    a2a_in = a2a_in1 if hh == 0 else a2a_in2
                    o_ps = ps2.tile([128, 512], F32, name="o_ps", tag="ops",
                                    bufs=1)
                    sum_ps = ps2.tile([128, 512], F32, name="sum_ps",
                                      tag="sums", bufs=1)
                    nkt = 4 * qsb + 4
                    for pr in range(nkt // 2):
                        kt0, kt1 = 2 * pr, 2 * pr + 1
                        jj0, jj1 = kt0 - 4 * qsb, kt1 - 4 * qsb
                        c00 = 0 if jj0 < 0 else jj0 * 128
                        c01 = 0 if jj1 < 0 else jj1 * 128
                        st = ps2.tile([128, 2, 512], F32, name="st",
                                      tag="stp", bufs=2)
                        mm = pe(nc.tensor.matmul(
                            st[:, 0, c00:512],
                            kTb[:, kt0 * 128:(kt0 + 1) * 128],
                            qT[:, qs + c00:qs + 512], start=True, stop=True))
                        if first_att_mm[0] is None:
                            first_att_mm[0] = mm
                        for p in pending:
                            add_dep_helper(mm.ins, p.ins, False)
                        pending.clear()
                        pe(nc.tensor.matmul(
                            st[:, 1, c01:512],
                            kTb[:, kt1 * 128:(kt1 + 1) * 128],
                            qT[:, qs + c01:qs + 512], start=True, stop=True))
                        pt_sb = att.tile([128, 2, 512], BF16, name="pt_sb",
                                         tag="ptp", bufs=4)
                        nc.scalar.activation(
                            pt_sb[:, :, c00:512], st[:, :, c00:512],
                            mybir.ActivationFunctionType.Exp, scale=SCALE)
                        if jj0 >= 0:
                            nc.vector.tensor_tensor(
                                pt_sb[:, 0, jj0 * 128:(jj0 + 1) * 128],
                                pt_sb[:, 0, jj0 * 128:(jj0 + 1) * 128],
                                triT[:], mybir.AluOpType.mult)
                            nc.vector.tensor_tensor(
                                pt_sb[:, 1, jj1 * 128:(jj1 + 1) * 128],
                                pt_sb[:, 1, jj1 * 128:(jj1 + 1) * 128],
                                triT[:], mybir.AluOpType.mult)
                        # pair-sum for the softmax denominator
                        padd = att.tile([128, 512], BF16, name="padd",
                                        tag="padd", bufs=2)
                        if jj0 < 0:
                            nc.vector.tensor_tensor(
                                padd[:], pt_sb[:, 0, :], pt_sb[:, 1, :],
                                mybir.AluOpType.add)
                        else:
                            nc.vector.tensor_copy(
                                padd[:, c00:c01], pt_sb[:, 0, c00:c01])
                            nc.vector.tensor_tensor(
                                padd[:, c01:512], pt_sb[:, 0, c01:512],
                                pt_sb[:, 1, c01:512], mybir.AluOpType.add)
                        pe(nc.tensor.matmul(
                            sum_ps[:, c00:512], ones_sb[:], padd[:, c00:512],
                            start=(pr == 0), stop=(pr == nkt // 2 - 1)))
                        pe(nc.tensor.matmul(
                            o_ps[:, c00:512], v_all[:, voff + kt0, :],
                            pt_sb[:, 0, c00:512], start=(kt0 == 0),
                            stop=False))
                        pe(nc.tensor.matmul(
                            o_ps[:, c01:512], v_all[:, voff + kt1, :],
                            pt_sb[:, 1, c01:512], start=False,
                            stop=(kt1 == nkt - 1)))

                    close_unit()
                    recip = att.tile([128, 512], F32, name="recip",
                                     tag="recip", bufs=2)
                    nc.vector.reciprocal_approx_fast(recip[:], sum_ps[:])
                    oT_sb = att.tile([128, 512], BF16, name="oT_sb",
                                     tag="osb", bufs=2)
                    nc.vector.scalar_tensor_tensor(
                        oT_sb[:], o_ps[:], 1.0, recip[:],
                        mybir.AluOpType.mult, mybir.AluOpType.mult)
                    d = nc.gpsimd.dma_start(a2a_in[b * 4 + qsb, :, :], oT_sb[:])
                    pending.append(d)

                # ---- emission schedule ----
                for pt in range(4):
                    q_proj(pt)
                units_h0 = [(0, 0, qsb) for qsb in range(4)]
                for i, u in enumerate(units_h0):
                    attn_unit(*u)
                    q_proj(4 + i)
                qstack.close()   # free x/rope/psq space before o_proj pools
                for qsb in range(4):
                    attn_unit(0, 1, qsb)
                nc.gpsimd.collective_compute(
                    "AllToAll", mybir.AluOpType.bypass,
                    replica_groups=[list(range(NCORES))],
                    ins=[a2a_in1.opt()], outs=[a2a_out1.opt()])

                # ====== Phase 3 part 1 (h0 contraction) interleaved ======
                with (
                    tc.tile_pool(name="proj", bufs=1) as proj,
                    tc.tile_pool(name="ps3", bufs=1, space="PSUM") as ps3,
                ):
                    for k4 in range(4):
                        wd = nc.scalar.dma_start(
                            wo_sb[:, k4 * 4:(k4 + 1) * 4, :],
                            wo_t[:, k4 * 4:(k4 + 1) * 4, :])
                        add_dep_helper(wd.ins, first_att_mm[0].ins, False)
                    for r in range(NCORES):
                        nc.gpsimd.dma_start(at1_sb[:, r, :], a2a_out1[r, :, :])
                    s1_sb = proj.tile([128, 16, 512], F32)

                    def part1_unit(ti):
                        mp, nn = ti // 4, ti % 4
                        ps_a = ps3.tile([128, 512], F32, name="ps_a",
                                        tag="po", bufs=2)
                        for r in range(NCORES):
                            pe(nc.tensor.matmul(
                                ps_a[:],
                                at1_sb[:, r, mp * 128:(mp + 1) * 128],
                                wo_sb[:, 2 * r, nn * 512:(nn + 1) * 512],
                                start=(r == 0), stop=(r == NCORES - 1)))
                        close_unit()
                        nc.vector.tensor_copy(s1_sb[:, ti, :], ps_a[:])

                    ti = 0
                    for b in range(B):
                        for qsb in range(4):
                            attn_unit(1, b, qsb)
                            part1_unit(ti)
                            part1_unit(ti + 1)
                            ti += 2
                    nc.gpsimd.collective_compute(
                        "AllToAll", mybir.AluOpType.bypass,
                        replica_groups=[list(range(NCORES))],
                        ins=[a2a_in2.opt()], outs=[a2a_out2.opt()])

                    # ====== Phase 3 part 2 (h1 contraction) + output =====
                    for r in range(NCORES):
                        nc.gpsimd.dma_start(at2_sb[:, r, :], a2a_out2[r, :, :])
                    for mp in range(RPC // 128):
                        for nn in range(H // 512):
                            ti = mp * 4 + nn
                            ps_b = ps3.tile([128, 512], F32, name="ps_b",
                                            tag="po", bufs=2)
                            for r in range(NCORES):
                                mm = pe(nc.tensor.matmul(
                                    ps_b[:],
                                    at2_sb[:, r, mp * 128:(mp + 1) * 128],
                                    wo_sb[:, 2 * r + 1,
                                          nn * 512:(nn + 1) * 512],
                                    start=(r == 0), stop=(r == NCORES - 1)))
                                for p in pending:
                                    add_dep_helper(mm.ins, p.ins, False)
                                pending.clear()
                            close_unit()
                            ev = proj.tile([128, 512], F32, name="ev",
                                           tag="ev", bufs=3)
                            nc.vector.scalar_tensor_tensor(
                                ev[:], ps_b[:], 1.0, s1_sb[:, ti, :],
                                mybir.AluOpType.mult, mybir.AluOpType.add)
                            nc.sync.dma_start(
                                out.ap()[mp * 128:(mp + 1) * 128,
                                         nn * 512:(nn + 1) * 512], ev[:])

    nc.compile()
    return nc


def _get_nc():
    if "nc" not in _CACHE:
        _CACHE["nc"] = _build()
    return _CACHE["nc"]


def _prep_inputs(x, cos, sin, wq, wk, wv, wo):
    bf = ml_dtypes.bfloat16
    xf = np.asarray(x, np.float32).reshape(POS, H)
    # [PT_N, KT, 128, 512]: xTt[pt,k,p,j] = x[pt*512+j, k*128+p]
    xT = np.ascontiguousarray(
        xf.reshape(PT_N, 512, KT, 128).transpose(0, 2, 3, 1)).astype(bf)
    cosT = np.ascontiguousarray(np.asarray(cos, np.float32).T).astype(bf)
    sinT = np.asarray(sin, np.float32).T.copy()
    sinT[0:64, :] = -sinT[0:64, :]
    sinT = np.ascontiguousarray(sinT).astype(bf)
    wo_b = np.ascontiguousarray(
        np.asarray(wo, np.float32).reshape(KT, 128, H)).astype(bf)
    wq = np.asarray(wq, np.float32)
    wk = np.asarray(wk, np.float32)
    wv = np.asarray(wv, np.float32)

    in_maps = []
    for i in range(NCORES):
        kv = i // 2
        half = i % 2
        in_maps.append({
            "xT": xT,
            "xkv": np.ascontiguousarray(xT[half * HPT:(half + 1) * HPT]),
            "wq": np.ascontiguousarray(
                wq[:, i * HPC * HD:(i + 1) * HPC * HD].reshape(
                    KT, 128, HPC * HD)).astype(bf),
            "wk": np.ascontiguousarray(
                wk[:, kv * HD:(kv + 1) * HD].reshape(KT, 128, HD)).astype(bf),
            "wv": np.ascontiguousarray(
                wv[:, kv * HD:(kv + 1) * HD].reshape(KT, 128, HD)).astype(bf),
            "cosT": cosT,
            "ssinT": sinT,
            "wo": wo_b,
        })
    return in_maps


def kernel(x, cos, sin, wq, wk, wv, wo, _trace=False):
    nc = _get_nc()
    in_maps = _prep_inputs(x, cos, sin, wq, wk, wv, wo)
    res = run_bass_kernel_spmd(nc, in_maps, core_ids=list(range(NCORES)),
                               trace=_trace)
    rows = np.concatenate([np.asarray(res.results[i]["out"])
                           for i in range(NCORES)], axis=0)
    out = rows.reshape(B, S, H).astype(np.float32)
    if _trace:
        _CACHE["last_exec_time_ns"] = res.exec_time_ns
        _CACHE["last_results"] = res
    return out


# revision 9
# speedup vs baseline: 1.7455x; 1.1415x over previous
"""Distributed GQA attention (B=2,S=2048,H=2048,NH=16,NKV=4,HD=128) on 8 TRN2 cores.

Strategy: tensor-parallel over heads (2 Q heads + 1 KV head per core).
K/V projections are split across core pairs by position (each core computes
its kv head's K and V for one batch only), exchanged with a pairwise
AllGather, with V produced transposed (512-wide matmuls) and fixed up by a
DMA transpose. Causal flash attention uses scores-transposed layout with
kt-paired exp instructions and pair-summed softmax denominators. An
AllToAll per head-half switches to sequence-parallel o_proj; the first
o_proj half is interleaved into the second attention half.
"""

import contextlib
import math

import numpy as np
import ml_dtypes

import concourse.bass as bass
import concourse.mybir as mybir
import concourse.tile as tile
from concourse.tile import add_dep_helper
from concourse import bacc
from concourse.bass_utils import run_bass_kernel_spmd
from concourse.masks import make_identity

BF16 = mybir.dt.bfloat16
F32 = mybir.dt.float32

B, S, H = 2, 2048, 2048
NH, NKV, HD = 16, 4, 128
NCORES = 8
HPC = NH // NCORES          # q heads per core = 2
POS = B * S                 # 4096 flattened rows
RPC = POS // NCORES         # output rows per core = 512
KT = H // 128               # 16 contraction tiles for projections
PT_N = POS // 512           # 8 pos-tiles of 512
HPT = PT_N // 2             # pos-tiles in my kv half = 4
SCALE = 1.0 / math.sqrt(HD)

_CACHE = {}


def _build():
    nc = bacc.Bacc("TRN2", target_bir_lowering=False, debug=False,
                   num_devices=NCORES)

    xT = nc.declare_dram_parameter("xT", [PT_N, KT, 128, 512], BF16,
                                   isOutput=False)
    xkv = nc.declare_dram_parameter("xkv", [HPT, KT, 128, 512], BF16,
                                    isOutput=False)
    wq = nc.declare_dram_parameter("wq", [KT, 128, HPC * HD], BF16,
                                   isOutput=False)
    wk = nc.declare_dram_parameter("wk", [KT, 128, HD], BF16, isOutput=False)
    wv = nc.declare_dram_parameter("wv", [KT, 128, HD], BF16, isOutput=False)
    cosT = nc.declare_dram_parameter("cosT", [HD, S], BF16, isOutput=False)
    ssinT = nc.declare_dram_parameter("ssinT", [HD, S], BF16, isOutput=False)
    wo = nc.declare_dram_parameter("wo", [KT, 128, H], BF16, isOutput=False)
    out = nc.declare_dram_parameter("out", [RPC, H], F32, isOutput=True)

    xT_t = xT.ap().rearrange("t k p n -> t p k n")
    xkv_t = xkv.ap().rearrange("t k p n -> t p k n")
    wq_t = wq.ap().rearrange("k p m -> p k m")
    wk_t = wk.ap().rearrange("k p m -> p k m")
    wv_t = wv.ap().rearrange("k p m -> p k m")
    wo_t = wo.ap().rearrange("k p m -> p k m")

    unit_last = [None]
    unit_first = [None]
    unit_latest = [None]

    def pe(mm):
        # chain PE work at unit granularity: the first matmul of each unit
        # depends on the last matmul of the previous unit; within a unit the
        # scheduler is free to pipeline.
        if unit_first[0] is None:
            unit_first[0] = mm
            if unit_last[0] is not None:
                add_dep_helper(mm.ins, unit_last[0].ins, False)
        unit_latest[0] = mm
        return mm

    def close_unit():
        unit_last[0] = unit_latest[0]
        unit_first[0] = None

    with tile.TileContext(nc) as tc:
        with (
            tc.tile_pool(name="const", bufs=1) as const,
            tc.tile_pool(name="wpool", bufs=1) as wpool,
            tc.tile_pool(name="qkv", bufs=1) as qkv,
            tc.tile_pool(name="dram", bufs=1, space="DRAM") as dram,
        ):
            # ---- constants / weights resident in SBUF ----
            ident = const.tile([128, 128], BF16)
            make_identity(nc, ident)
            # upper-triangular (incl diag) mask: valid where kpos <= q
            triT = const.tile([128, 128], BF16)
            nc.gpsimd.memset(triT, 1.0)
            nc.gpsimd.affine_select(
                out=triT, in_=triT, compare_op=mybir.AluOpType.is_ge,
                fill=0.0, base=0, pattern=[[1, 128]], channel_multiplier=-1,
            )  # keep 1.0 where (c - p) >= 0, i.e. kpos <= q
            ones_sb = const.tile([128, 128], BF16)
            nc.gpsimd.memset(ones_sb, 1.0)

            cos_sb = const.tile([128, S], BF16)
            sin_sb = const.tile([128, S], BF16)
            cs_dmas = [nc.scalar.dma_start(cos_sb[:], cosT.ap()),
                       nc.scalar.dma_start(sin_sb[:], ssinT.ap())]

            wq_sb = wpool.tile([128, KT, HPC * HD], BF16)
            wk_sb = wpool.tile([128, KT, HD], BF16)
            wv_sb = wpool.tile([128, KT, HD], BF16)
            nc.scalar.dma_start(wk_sb[:], wk_t)
            nc.scalar.dma_start(wv_sb[:], wv_t)
            nc.scalar.dma_start(wq_sb[:], wq_t)
            wo_sb = wpool.tile([128, KT, H], BF16)
            at1_sb = wpool.tile([128, NCORES, RPC], BF16)
            at2_sb = wpool.tile([128, NCORES, RPC], BF16)

            # persistent q/k/v for both batches (bf16)
            q_all = qkv.tile([128, HPC, POS], BF16)
            kT_all = qkv.tile([128, POS], BF16)
            v_all = qkv.tile([128, POS // 128, HD], BF16)

            exch_in = dram.tile([128, 4096], BF16)
            exch_out = dram.tile([2, 128, 4096], BF16)
            a2a_in1 = dram.tile([NCORES, HD, RPC], BF16)
            a2a_out1 = dram.tile([NCORES, HD, RPC], BF16)
            a2a_in2 = dram.tile([NCORES, HD, RPC], BF16)
            a2a_out2 = dram.tile([NCORES, HD, RPC], BF16)

            # ---- PE warmup: flip HAM to K=8/8 before real matmuls ----
            with tc.tile_pool(name="psw", bufs=1, space="PSUM") as psw:
                ps_w = psw.tile([128, 128], F32, name="ps_w")
                for _ in range(90):
                    pe(nc.tensor.matmul(ps_w[:], ident[:], ident[:],
                                        start=True, stop=True))
                close_unit()

            def rope(dst, ps, c0, rope_pool):
                """dst[128,512] bf16 = ps*cos + swap_halves(ps)*ssin."""
                ra = rope_pool.tile([128, 512], BF16, name="ra", tag="ra",
                                    bufs=3)
                rb = rope_pool.tile([128, 512], BF16, name="rb", tag="rb",
                                    bufs=3)
                nc.vector.tensor_tensor(
                    ra[:], ps[:], cos_sb[:, c0:c0 + 512], mybir.AluOpType.mult)
                nc.vector.tensor_tensor(
                    rb[0:64, :], ps[64:128, :], sin_sb[0:64, c0:c0 + 512],
                    mybir.AluOpType.mult)
                nc.vector.tensor_tensor(
                    rb[64:128, :], ps[0:64, :], sin_sb[64:128, c0:c0 + 512],
                    mybir.AluOpType.mult)
                nc.vector.tensor_tensor(dst, ra[:], rb[:],
                                        mybir.AluOpType.add)

            # ====== Phase A: K/V projection on my half + pair exchange ====
            with (
                tc.tile_pool(name="kvtiles", bufs=1) as kvtiles,
                tc.tile_pool(name="kvout", bufs=1) as kvout,
                tc.tile_pool(name="ropeA", bufs=1) as ropeA,
                tc.tile_pool(name="psA", bufs=1, space="PSUM") as psA,
            ):
                kTh = kvout.tile([128, S], BF16)
                vTh = kvout.tile([128, S], BF16)
                vh = kvout.tile([128, S // 128, HD], BF16)
                for pt in range(HPT):
                    c0 = pt * 512
                    xk_t = kvtiles.tile([128, KT, 512], BF16, name="xk_t",
                                        tag="xk", bufs=2)
                    for k8 in range(8):
                        xd = nc.sync.dma_start(
                            xk_t[:, k8 * 2:(k8 + 1) * 2, :],
                            xkv_t[pt, :, k8 * 2:(k8 + 1) * 2, :])
                    if pt == 0:
                        for csd in cs_dmas:
                            add_dep_helper(csd.ins, xd.ins, False)
                    ps_k = psA.tile([128, 512], F32, name="ps_k", tag="psk",
                                    bufs=2)
                    for k in range(KT):
                        pe(nc.tensor.matmul(ps_k[:], wk_sb[:, k, :],
                                            xk_t[:, k, :], start=(k == 0),
                                            stop=(k == KT - 1)))
                    close_unit()
                    rope(kTh[:, pt * 512:(pt + 1) * 512], ps_k, c0, ropeA)
                    ps_v = psA.tile([128, 512], F32, name="ps_v", tag="psv",
                                    bufs=2)
                    for k in range(KT):
                        pe(nc.tensor.matmul(ps_v[:], wv_sb[:, k, :],
                                            xk_t[:, k, :], start=(k == 0),
                                            stop=(k == KT - 1)))
                    close_unit()
                    nc.scalar.copy(vTh[:, pt * 512:(pt + 1) * 512], ps_v[:])

                # v: [d, pos] -> [pos%128, pos//128, d] via DMA transpose
                nc.scalar.dma_start_transpose(vh[:], vTh[:])
                nc.scalar.dma_start(exch_in[:, 0:2048], kTh[:])
                nc.scalar.dma_start(exch_in[:, 2048:4096], vh[:])
                nc.gpsimd.collective_compute(
                    "AllGather", mybir.AluOpType.bypass,
                    replica_groups=[[0, 1], [2, 3], [4, 5], [6, 7]],
                    ins=[exch_in.opt()], outs=[exch_out.opt()])
                nc.scalar.dma_start(kT_all[:, 0:2048], exch_out[0, :, 0:2048])
                nc.scalar.dma_start(kT_all[:, 2048:4096],
                                  exch_out[1, :, 0:2048])
                nc.scalar.dma_start(v_all[:, 0:16, :], exch_out[0, :, 2048:4096])
                nc.scalar.dma_start(v_all[:, 16:32, :],
                                  exch_out[1, :, 2048:4096])

            # ====== Phase B: Q projection (global order) =================
            with (
                tc.tile_pool(name="att", bufs=1) as att,
                tc.tile_pool(name="ps2", bufs=1, space="PSUM") as ps2,
            ):
                qstack = contextlib.ExitStack()
                xtiles = qstack.enter_context(
                    tc.tile_pool(name="xtiles", bufs=1))
                ropeB = qstack.enter_context(tc.tile_pool(name="ropeB", bufs=1))
                psB = qstack.enter_context(
                    tc.tile_pool(name="psB", bufs=1, space="PSUM"))
                pending = []
                first_att_mm = [None]

                def q_proj(pt):
                    c0 = (pt * 512) % S
                    x_t = xtiles.tile([128, KT, 512], BF16, name="x_t",
                                      tag="x", bufs=2)
                    for k4 in range(4):
                        nc.sync.dma_start(
                            x_t[:, k4 * 4:(k4 + 1) * 4, :],
                            xT_t[pt, :, k4 * 4:(k4 + 1) * 4, :])
                    for hh in range(HPC):
                        ps_q = psB.tile([128, 512], F32, name="ps_q",
                                        tag="psq", bufs=2)
                        for k in range(KT):
                            mm = pe(nc.tensor.matmul(
                                ps_q[:], wq_sb[:, k, hh * 128:(hh + 1) * 128],
                                x_t[:, k, :], start=(k == 0),
                                stop=(k == KT - 1)))
                            for p in pending:
                                add_dep_helper(mm.ins, p.ins, False)
                            pending.clear()
                        close_unit()
                        rope(q_all[:, hh, pt * 512:(pt + 1) * 512], ps_q,
                             c0, ropeB)

                # ====== attention unit (ST flash, kt-paired exp) =========
                def attn_unit(hh, b, qsb):
                    qT = q_all[:, hh, b * S:(b + 1) * S]
                    kTb = kT_all[:, b * S:(b + 1) * S]
                    voff = b * (S // 128)
                    qs = qsb * 512
                    a2a_in = a2a_in1 if hh == 0 else a2a_in2
                    o_ps = ps2.tile([128, 512], F32, name="o_ps", tag="ops",
                                    bufs=1)
                    sum_ps = ps2.tile([128, 512], F32, name="sum_ps",
                                      tag="sums", bufs=1)
                    nkt = 4 * qsb + 4
                    for pr in range(nkt // 2):
                        kt0, kt1 = 2 * pr, 2 * pr + 1
                        jj0, jj1 = kt0 - 4 * qsb, kt1 - 4 * qsb
                        c00 = 0 if jj0 < 0 else jj0 * 128
                        c01 = 0 if jj1 < 0 else jj1 * 128
                        st = ps2.tile([128, 2, 512], F32, name="st",
                                      tag="stp", bufs=2)
                        mm = pe(nc.tensor.matmul(
                            st[:, 0, c00:512],
                            kTb[:, kt0 * 128:(kt0 + 1) * 128],
                            qT[:, qs + c00:qs + 512], start=True, stop=True))
                        if first_att_mm[0] is None:
                            first_att_mm[0] = mm
                        for p in pending:
                            add_dep_helper(mm.ins, p.ins, False)
                        pending.clear()
                        pe(nc.tensor.matmul(
                            st[:, 1, c01:512],
                            kTb[:, kt1 * 128:(kt1 + 1) * 128],
                            qT[:, qs + c01:qs + 512], start=True, stop=True))
                        pt_sb = att.tile([128, 2, 512], BF16, name="pt_sb",
                                         tag="ptp", bufs=4)
                        nc.scalar.activation(
                            pt_sb[:, :, c00:512], st[:, :, c00:512],
                            mybir.ActivationFunctionType.Exp, scale=SCALE)
                        if jj0 >= 0:
                            nc.vector.tensor_tensor(
                                pt_sb[:, 0, jj0 * 128:(jj0 + 1) * 128],
                                pt_sb[:, 0, jj0 * 128:(jj0 + 1) * 128],
                                triT[:], mybir.AluOpType.mult)
                            nc.vector.tensor_tensor(
                                pt_sb[:, 1, jj1 * 128:(jj1 + 1) * 128],
                                pt_sb[:, 1, jj1 * 128:(jj1 + 1) * 128],
                                triT[:], mybir.AluOpType.mult)
                        # pair-sum for the softmax denominator
                        padd = att.tile([128, 512], BF16, name="padd",
                                        tag="padd", bufs=2)
                        if jj0 < 0:
                            nc.vector.tensor_tensor(
                                padd[:], pt_sb[:, 0, :], pt_sb[:, 1, :],
                                mybir.AluOpType.add)
                        else:
                            nc.vector.tensor_copy(
                                padd[:, c00:c01], pt_sb[:, 0, c00:c01])
                            nc.vector.tensor_tensor(
                                padd[:, c01:512], pt_sb[:, 0, c01:512],
                                pt_sb[:, 1, c01:512], mybir.AluOpType.add)
                        pe(nc.tensor.matmul(
                            sum_ps[:, c00:512], ones_sb[:], padd[:, c00:512],
                            start=(pr == 0), stop=(pr == nkt // 2 - 1)))
                        pe(nc.tensor.matmul(
                            o_ps[:, c00:512], v_all[:, voff + kt0, :],
                            pt_sb[:, 0, c00:512], start=(kt0 == 0),
                            stop=False))
                        pe(nc.tensor.matmul(
                            o_ps[:, c01:512], v_all[:, voff + kt1, :],
                            pt_sb[:, 1, c01:512], start=False,
                            stop=(kt1 == nkt - 1)))

                    close_unit()
                    recip = att.tile([128, 512], F32, name="recip",
                                     tag="recip", bufs=2)
                    nc.vector.reciprocal_approx_fast(recip[:], sum_ps[:])
                    oT_sb = att.tile([128, 512], BF16, name="oT_sb",
                                     tag="osb", bufs=2)
                    nc.vector.scalar_tensor_tensor(
                        oT_sb[:], o_ps[:], 1.0, recip[:],
                        mybir.AluOpType.mult, mybir.AluOpType.mult)
                    d = nc.sync.dma_start(a2a_in[b * 4 + qsb, :, :], oT_sb[:])
                    pending.append(d)

                # ---- emission schedule ----
                for pt in range(PT_N):
                    q_proj(pt)
                qstack.close()   # free x/rope/psq space before o_proj pools
                for b in range(B):
                    for qsb in range(4):
                        attn_unit(0, b, qsb)
                nc.gpsimd.collective_compute(
                    "AllToAll", mybir.AluOpType.bypass,
                    replica_groups=[list(range(NCORES))],
                    ins=[a2a_in1.opt()], outs=[a2a_out1.opt()])

                # ====== Phase 3 part 1 (h0 contraction) interleaved ======
                with (
                    tc.tile_pool(name="proj", bufs=1) as proj,
                    tc.tile_pool(name="ps3", bufs=1, space="PSUM") as ps3,
                ):
                    for k4 in range(4):
                        wd = nc.scalar.dma_start(
                            wo_sb[:, k4 * 4:(k4 + 1) * 4, :],
                            wo_t[:, k4 * 4:(k4 + 1) * 4, :])
                        add_dep_helper(wd.ins, first_att_mm[0].ins, False)
                    for r in range(NCORES):
                        nc.sync.dma_start(at1_sb[:, r, :], a2a_out1[r, :, :])
                    s1_sb = proj.tile([128, 16, 512], F32)

                    def part1_unit(ti):
                        mp, nn = ti // 4, ti % 4
                        ps_a = ps3.tile([128, 512], F32, name="ps_a",
                                        tag="po", bufs=2)
                        for r in range(NCORES):
                            pe(nc.tensor.matmul(
                                ps_a[:],
                                at1_sb[:, r, mp * 128:(mp + 1) * 128],
                                wo_sb[:, 2 * r, nn * 512:(nn + 1) * 512],
                                start=(r == 0), stop=(r == NCORES - 1)))
                        close_unit()
                        nc.vector.tensor_copy(s1_sb[:, ti, :], ps_a[:])

                    for b in range(B):
                        for qsb in range(4):
                            attn_unit(1, b, qsb)
                    for ti in range(16):
                        part1_unit(ti)
                    nc.gpsimd.collective_compute(
                        "AllToAll", mybir.AluOpType.bypass,
                        replica_groups=[list(range(NCORES))],
                        ins=[a2a_in2.opt()], outs=[a2a_out2.opt()])

                    # ====== Phase 3 part 2 (h1 contraction) + output =====
                    for r in range(NCORES):
                        nc.sync.dma_start(at2_sb[:, r, :], a2a_out2[r, :, :])
                    for mp in range(RPC // 128):
                        for nn in range(H // 512):
                            ti = mp * 4 + nn
                            ps_b = ps3.tile([128, 512], F32, name="ps_b",
                                            tag="po", bufs=2)
                            for r in range(NCORES):
                                mm = pe(nc.tensor.matmul(
                                    ps_b[:],
                                    at2_sb[:, r, mp * 128:(mp + 1) * 128],
                                    wo_sb[:, 2 * r + 1,
                                          nn * 512:(nn + 1) * 512],
                                    start=(r == 0), stop=(r == NCORES - 1)))
                                for p in pending:
                                    add_dep_helper(mm.ins, p.ins, False)
                                pending.clear()
                            close_unit()
                            ev = proj.tile([128, 512], F32, name="ev",
                                           tag="ev", bufs=3)
                            nc.vector.scalar_tensor_tensor(
                                ev[:], ps_b[:], 1.0, s1_sb[:, ti, :],
                                mybir.AluOpType.mult, mybir.AluOpType.add)
                            nc.sync.dma_start(
                                out.ap()[mp * 128:(mp + 1) * 128,
                                         nn * 512:(nn + 1) * 512], ev[:])

    nc.compile()
    return nc


def _get_nc():
    if "nc" not in _CACHE:
        _CACHE["nc"] = _build()
    return _CACHE["nc"]


def _prep_inputs(x, cos, sin, wq, wk, wv, wo):
    bf = ml_dtypes.bfloat16
    xf = np.asarray(x, np.float32).reshape(POS, H)
    # [PT_N, KT, 128, 512]: xTt[pt,k,p,j] = x[pt*512+j, k*128+p]
    xT = np.ascontiguousarray(
        xf.reshape(PT_N, 512, KT, 128).transpose(0, 2, 3, 1)).astype(bf)
    cosT = np.ascontiguousarray(np.asarray(cos, np.float32).T).astype(bf)
    sinT = np.asarray(sin, np.float32).T.copy()
    sinT[0:64, :] = -sinT[0:64, :]
    sinT = np.ascontiguousarray(sinT).astype(bf)
    wo_b = np.ascontiguousarray(
        np.asarray(wo, np.float32).reshape(KT, 128, H)).astype(bf)
    wq = np.asarray(wq, np.float32)
    wk = np.asarray(wk, np.float32)
    wv = np.asarray(wv, np.float32)

    in_maps = []
    for i in range(NCORES):
        kv = i // 2
        half = i % 2
        in_maps.append({
            "xT": xT,
            "xkv": np.ascontiguousarray(xT[half * HPT:(half + 1) * HPT]),
            "wq": np.ascontiguousarray(
                wq[:, i * HPC * HD:(i + 1) * HPC * HD].reshape(
                    KT, 128, HPC * HD)).astype(bf),
            "wk": np.ascontiguousarray(
                wk[:, kv * HD:(kv + 1) * HD].reshape(KT, 128, HD)).astype(bf),
            "wv": np.ascontiguousarray(
                wv[:, kv * HD:(kv + 1) * HD].reshape(KT, 128, HD)).astype(bf),
            "cosT": cosT,
            "ssinT": sinT,
            "wo": wo_b,
        })
    return in_maps


def kernel(x, cos, sin, wq, wk, wv, wo, _trace=False):
    nc = _get_nc()
    in_maps = _prep_inputs(x, cos, sin, wq, wk, wv, wo)
    res = run_bass_kernel_spmd(nc, in_maps, core_ids=list(range(NCORES)),
                               trace=_trace)
    rows = np.concatenate([np.asarray(res.results[i]["out"])
                           for i in range(NCORES)], axis=0)
    out = rows.reshape(B, S, H).astype(np.float32)
    if _trace:
        _CACHE["last_exec_time_ns"] = res.exec_time_ns
        _CACHE["last_results"] = res
    return out
